# revision 42
# baseline (speedup 1.0000x reference)
"""Trainium2 Bass kernel for nn_Attention_34187939676584.

Per-core shard: one batch element + one pair of heads (8 cores = 2 batches x 4
head-pairs).  Inside one core (all fp32; float32r streaming for matmuls):

  phase A: RMSNorm   xn = x * s0,  s0 = sqrt(C)/||x||_c   ((gamma+1) is folded
           into the qkv weights on the host)
  phase B: qkv projection.  q,k produced in [d, pos] layout (d on partitions),
           v produced directly transposed [pos, d] with a ones-column appended
           so the PV matmul also computes the softmax denominators.
           mem_kv tokens are appended at the END of the kv sequence (softmax
           is order-invariant), padded with zeros to a multiple of 128.
  phase C: attention.  simT[j, i] = k^T q computed with the two heads packed
           into the 128-row systolic array (K=64 row groups).  exp() on the
           scalar engine straight out of PSUM with the 1/8 scale folded in
           (no max-subtraction: |sim|*0.125 stays small).  PV accumulates
           out2[65, i] = [v|1]^T P over all j chunks in PSUM.
  phase D: output projection (K=64 per head, accumulated), DMA out.

The sim-PSUM rotation is 3 tiles deep (ps_qk bufs=3): the steady-state
binding loop is QK(t) -> exp(t) -> slot free -> QK(t+3), so a 2-deep
rotation caps the chunk rate at (QK + exp + 2 sem hops)/2 ~ 1.1us while
the PE's per-chunk work is only ~0.85us.  The two PSUM banks for the
third slot come from running the out2 accumulators single-buffered
(norm_a copies emitted immediately at each block boundary) and moving
the ptq q-prefetch into the qk tag rotation.  Further: phase-A squares
and the q/k psum->sbuf copies run on the Vector engine (keeps the
ACT-bound exp stream clean), and phase-B projection emission is
interleaved just-in-time into block 0's attention loop so the exp
stream starts ~30us earlier.  Measured on HW (For_i trip-count sweep):
458 -> 376 -> 369 -> 365 us/iter.

The host sums the 4 head-pair partial outputs per batch (the "all-reduce").
"""

import numpy as np

import concourse.bass as bass
import concourse.bacc as bacc
import concourse.mybir as mybir
import concourse.tile as tile
from concourse import bass_utils

F32 = mybir.dt.float32
F32R = mybir.dt.float32r
AF = mybir.ActivationFunctionType

HEADS = 8
DIM_HEAD = 64
C = 256
LN16 = float(np.log(16.0))


def _r(ap):
    """fp32 -> float32r view: same bits, 4x faster matmul streaming (N>=256)."""
    return ap.bitcast(F32R)


def round_f32r(a):
    """Round fp32 ndarray to FP32R (e8m11: low 12 mantissa bits zero),
    nearest-even — matches walrus fp32_to_fp32r."""
    b = np.ascontiguousarray(a, np.float32).view(np.uint32)
    b = (b + 0x7FF + ((b >> 12) & 1)) & 0xFFFFF000
    return b.view(np.float32)


def emit_kernel(tc, nc, io, NPOS=4096):
    """Emit the per-core program.  io: dict name -> AP (dram)."""
    assert NPOS % 512 == 0
    NPC = NPOS // 128          # position chunks of 128
    NJC = NPC + 1              # + one chunk holding the 4 mem tokens (zero pad)
    KW = NJC * 128             # padded kv length
    NN = NPOS // 512           # 512-wide chunks of the position axis
    NIB = NPOS // 512          # i-blocks for attention

    x_d = io["x_sh"]           # [256, NPOS]
    w_d = io["wqkvT"]          # [256, 512]  (q_h0|q_h1|k_h0|k_h1|v_h0|v_h1|pad)
    kki_d = io["kkinit"]       # [128, 128] (cols 0:4 = mem_k, rest zero)
    vvi_d = io["vvtinit"]      # [128, NJC, 130] (ones cols, mem_v chunk, zeros)
    wo_d = io["wout"]          # [64, 512]   (h0: cols 0:256, h1: cols 256:512)
    y_d = io["y"]              # [256, NPOS]

    from contextlib import ExitStack

    with ExitStack() as ctx:
        main = ctx.enter_context(tc.tile_pool(name="main", bufs=1))

        qq = main.tile([128, NPOS], F32R)
        kk = main.tile([128, KW], F32R)
        vvt = main.tile([128, NJC, 130], F32R)
        on_h0 = main.tile([64, NPOS], F32R)
        on_h1 = main.tile([64, NPOS], F32R)
        w1 = main.tile([128, 512], F32R)
        w2 = main.tile([128, 512], F32R)
        wo = main.tile([64, 512], F32R)
        ones128 = w_d  # placeholder; real ones column lives in w1[:, 384]

        nc.sync.dma_start(w1[:], w_d[0:128, :])
        nc.sync.dma_start(w2[:], w_d[128:256, :])

        # ---------------- phase A: load x, RMS norm ----------------
        pa = ctx.enter_context(tc.tile_pool(name="ph_a", bufs=1))
        pa_scope = ExitStack()
        par = pa_scope.enter_context(tc.tile_pool(name="ph_a_rot", bufs=3))
        with tc.tile_pool(name="ps_a", bufs=2, space="PSUM") as psa:
            xa = par.tile([128, NPOS], F32R, tag="xa", bufs=1)
            xb = par.tile([128, NPOS], F32R, tag="xb", bufs=1)
            s0c = {}
            # pass 1: stream x chunks for the sum-of-squares only;
            # s0 holds sqrt(sumsq)/16 chunks (ACT Sqrt is the only phase-A
            # table function, so it never thrashes against phase C's Exp)
            for n in range(NN):
                s = bass.ts(n, 512)
                xr = par.tile([128, 2, 512], F32, tag="xr", bufs=3)
                nc.sync.dma_start(xr[:, 0, :], x_d[0:128, s])
                nc.sync.dma_start(xr[:, 1, :], x_d[128:256, s])
                xsq = par.tile([128, 2, 512], F32R, tag="xsq", bufs=2)
                nc.vector.tensor_mul(xsq[:, 0, :], xr[:, 0, :],
                                     xr[:, 0, :])
                nc.vector.tensor_mul(xsq[:, 1, :], xr[:, 1, :],
                                     xr[:, 1, :])
                ss = psa.tile([1, 512], F32, tag="ss")
                nc.tensor.matmul(ss[:], w1[:, 384:385], _r(xsq[:, 0, :]),
                                 start=True, stop=False)
                nc.tensor.matmul(ss[:], w1[:, 384:385], _r(xsq[:, 1, :]),
                                 start=False, stop=True)
                s0n = par.tile([1, 512], F32, tag="s0", bufs=4, name="s0n")
                nc.scalar.activation(s0n[:], ss[:], AF.Sqrt,
                                     scale=1.0 / 256.0)
                s0c[n] = s0n
            # pass 2: re-stream x, normalize into the f32r xn tiles;
            # per-chunk s0 = 1/(sqrt(u)/16) via the 2-ULP DVE reciprocal
            for n in range(NN):
                s = bass.ts(n, 512)
                xr = par.tile([128, 2, 512], F32, tag="xr2", bufs=2)
                nc.sync.dma_start(xr[:, 0, :], x_d[0:128, s])
                nc.sync.dma_start(xr[:, 1, :], x_d[128:256, s])
                rc = par.tile([1, 512], F32, tag="rc", bufs=2)
                rcs = par.tile([1, 512], F32, tag="rcs", bufs=2)
                nc.vector.reciprocal_approx_accurate(rc[:], s0c[n][:],
                                                     rcs[:])
                s0b = par.tile([128, 512], F32, tag="s0b")
                nc.gpsimd.partition_broadcast(s0b[:], rc[:])
                nc.vector.tensor_mul(xa[:, s], xr[:, 0, :], s0b[:])
                nc.vector.tensor_mul(xb[:, s], xr[:, 1, :], s0b[:])

        # big init DMAs queued after the x loads so phase A starts sooner
        nc.sync.dma_start(wo[:], wo_d[:, :])
        # kv tail: mem_k tokens + zero padding, host-prepared
        nc.sync.dma_start(kk[:, NPOS:KW], kki_d[:, :])
        # vvt skeleton: ones columns, mem_v chunk, zeros (v slots DMA'd over)
        nc.sync.dma_start(vvt[:, :, :], vvi_d[:, :, :])

        # ------- phases B + C share psum pools so Tile can overlap -------
        with tc.tile_pool(name="ps_qk", bufs=3, space="PSUM") as psqk, \
             tc.tile_pool(name="ps_pv", bufs=1, space="PSUM") as pspv, \
             tc.tile_pool(name="ph_c", bufs=4) as pc, \
             tc.tile_pool(name="ph_c1", bufs=1) as pc1:
            # ---------------- phase B: qkv projection ----------------
            def proj_qk(n, col, dst):
                s = bass.ts(n, 512)
                pt = psqk.tile([128, 1024], F32, tag="qk", name="pt")
                nc.tensor.matmul(pt[:, 0:512], _r(w1[:, col:col + 128]),
                                 _r(xa[:, s]), start=True, stop=False)
                nc.tensor.matmul(pt[:, 0:512], _r(w2[:, col:col + 128]),
                                 _r(xb[:, s]), start=False, stop=True)
                nc.vector.tensor_copy(dst[:, s], pt[:, 0:512])

            def proj_b(n):
                # k/v projections for position chunk n (kv j-chunks
                # 4n..4n+3); emitted just-in-time from inside block 0's
                # attention loop so the exp stream starts ~30us earlier
                proj_qk(n, 128, kk)
                for m in range(4 * n, 4 * n + 4):
                    sp = bass.ts(m, 128)
                    pv = psqk.tile([128, 1024], F32, tag="qk", name="pv")
                    nc.tensor.matmul(pv[:, 0:256], _r(xa[:, sp]),
                                     _r(w1[:, 256:512]), start=True, stop=False)
                    nc.tensor.matmul(pv[:, 0:256], _r(xb[:, sp]),
                                     _r(w2[:, 256:512]), start=False, stop=True)
                    # one strided copy: psum [h0|h1] -> vvt cols {0:64, 65:129}
                    dst3 = vvt[:, m, 0:130].rearrange(
                        "p (g c) -> p g c", g=2, c=65)[:, :, 0:64]
                    src3 = pv[:, 0:128].rearrange("p (g c) -> p g c", g=2, c=64)
                    nc.vector.tensor_copy(dst3, src3)

            proj_qk(0, 0, qq)
            proj_b(0)

            def proj_qq_late(n):
                # prefetch block n's q columns one block ahead, off the
                # contended qk slots (o2a-tag bank has slack mid-block)
                s = bass.ts(n, 512)
                ptq = psqk.tile([128, 1024], F32, tag="qk", name="ptq")[:, 0:512]
                nc.tensor.matmul(ptq[:], _r(w1[:, 0:128]), _r(xa[:, s]),
                                 start=True, stop=False)
                nc.tensor.matmul(ptq[:], _r(w2[:, 0:128]), _r(xb[:, s]),
                                 start=False, stop=True)
                nc.vector.tensor_copy(qq[:, s], ptq[:])

            # ---------------- phase C: attention ----------------
            def qk_pair(si, jc):
                sj = bass.ts(jc, 128)
                ps = psqk.tile([128, 1024], F32, tag="qk", name="ps")
                nc.tensor.matmul(ps[:, 0:512], _r(kk[0:64, sj]),
                                 _r(qq[0:64, si]), start=True, stop=True)
                nc.tensor.matmul(ps[:, 512:1024], _r(kk[64:128, sj]),
                                 _r(qq[64:128, si]), start=True, stop=True)
                return ps

            def epilogue_steps(ib, out2_h0, out2_h1):
                """Normalize block ib's accumulators + output projection,
                as a list of thunks spread across block ib+1 to fill slack."""
                si = bass.ts(ib, 512)
                state = {}

                def norm_a(out2, on_h, key):
                    def f():
                        o2 = pc1.tile([65, 512], F32, tag="o2s", bufs=2,
                                      name="o2")
                        nc.vector.tensor_copy(o2[:], out2[:])
                        state[key] = o2
                    return f

                def norm_b(on_h, key):
                    def f():
                        o2 = state[key]
                        # custom-DVE ops misread nonzero base partitions on
                        # HW; hop the sums row to partition 0 via DMA first
                        sc = pc1.tile([1, 512], F32, tag="sc", name="sc")
                        nc.sync.dma_start(sc[:], o2[64:65, :])
                        r = pc1.tile([1, 512], F32, tag="r", name="r")
                        rs = pc1.tile([1, 512], F32, tag="rs", name="rs")
                        nc.vector.reciprocal_approx_accurate(
                            r[:], sc[:], rs[:])
                        sb = pc1.tile([64, 512], F32, tag="sb", name="sb")
                        nc.gpsimd.partition_broadcast(sb[:], r[:])
                        nc.vector.tensor_mul(on_h[:, si], o2[0:64, :], sb[:])
                    return f

                def proj_y(mt, rows, tag):
                    def f():
                        py = pspv.tile([128, 512], F32, tag=tag, name="py")
                        nc.tensor.matmul(py[:], _r(wo[:, bass.ts(mt, 128)]),
                                         _r(on_h0[:, si]), start=True,
                                         stop=False)
                        nc.tensor.matmul(
                            py[:], _r(wo[:, 256 + mt * 128:384 + mt * 128]),
                            _r(on_h1[:, si]), start=False, stop=True)
                        y_sb = pc1.tile([128, 512], F32, tag="ysb", bufs=2,
                                        name="y_sb")
                        nc.vector.tensor_copy(y_sb[:], py[:])
                        nc.sync.dma_start(y_d[rows, si], y_sb[:])
                    return f

                return [norm_a(out2_h0, on_h0, "a"), norm_a(out2_h1, on_h1, "b"),
                        norm_b(on_h0, "a"), norm_b(on_h1, "b"),
                        proj_y(0, slice(0, 128), "o2a"),
                        proj_y(1, slice(128, 256), "o2b")]

            # flat software pipeline over all (ib, jc): QK issued one step
            # ahead so the exp stream never waits on trailing PV matmuls,
            # including across i-block boundaries.
            items = [(ib, jc) for ib in range(NIB) for jc in range(NJC)]
            out2 = {}
            pending = None

            def qk_step(t):
                ib, jc = items[t]
                if jc == 0:
                    out2[ib] = (pspv.tile([65, 512], F32, tag="o2a",
                                          name="o2t0"),
                                pspv.tile([65, 512], F32, tag="o2b",
                                          name="o2t1"))
                return qk_pair(bass.ts(ib, 512), jc)

            ps_cur = qk_step(0)
            steps = []
            for t, (ib, jc) in enumerate(items):
                if (ib == 0 and (jc + 2) % 4 == 0
                        and 1 <= (jc + 2) // 4 < NN):
                    proj_b((jc + 2) // 4)
                ps_next = qk_step(t + 1) if t + 1 < len(items) else None
                if jc == 0:
                    if ib + 1 < NIB:
                        proj_qq_late(ib + 1)
                    if pending is not None:
                        for f in steps:  # flush leftovers of older blocks
                            f()
                        steps = epilogue_steps(*pending)
                        pending = None
                        steps.pop(0)()   # norm_a h0: frees the o2a slot
                        steps.pop(0)()   # norm_a h1: frees the o2b slot
                if steps and jc % max(1, (NJC - 1) // 6) == 1:
                    steps.pop(0)()
                pt2 = pc.tile([128, 1024], F32R, tag="pt2")
                nc.scalar.activation(pt2[:], ps_cur[:], AF.Exp, scale=0.125)
                out2_h0, out2_h1 = out2[ib]
                nc.tensor.matmul(out2_h0[:], _r(vvt[:, jc, 0:65]),
                                 _r(pt2[:, 0:512]), start=(jc == 0),
                                 stop=(jc == NJC - 1), skip_group_check=True)
                nc.tensor.matmul(out2_h1[:], _r(vvt[:, jc, 65:130]),
                                 _r(pt2[:, 512:1024]), start=(jc == 0),
                                 stop=(jc == NJC - 1), skip_group_check=True)
                if jc == NJC - 1:
                    pending = (ib, out2_h0, out2_h1)
                ps_cur = ps_next
            for f in steps:
                f()
            for f in epilogue_steps(*pending):
                f()

        pa_scope.close()


def build_nc(NPOS=4096, debug=False, loop_iters=None):
    nc = bacc.Bacc("TRN2", target_bir_lowering=False, debug=debug,
                   num_devices=8)
    io = {
        "x_sh": nc.dram_tensor("x_sh", [256, NPOS], F32,
                               kind="ExternalInput").ap(),
        "wqkvT": nc.dram_tensor("wqkvT", [256, 512], F32R,
                                kind="ExternalInput").ap(),
        "kkinit": nc.dram_tensor("kkinit", [128, 128], F32R,
                                 kind="ExternalInput").ap(),
        "vvtinit": nc.dram_tensor("vvtinit", [128, NPOS // 128 + 1, 130], F32R,
                                  kind="ExternalInput").ap(),
        "wout": nc.dram_tensor("wout", [64, 512], F32R,
                               kind="ExternalInput").ap(),
        "y": nc.dram_tensor("y", [256, NPOS], F32,
                            kind="ExternalOutput").ap(),
    }
    with tile.TileContext(nc) as tc:
        if loop_iters is not None:
            with tc.For_i(0, loop_iters, 1):
                emit_kernel(tc, nc, io, NPOS=NPOS)
        else:
            emit_kernel(tc, nc, io, NPOS=NPOS)
    nc.compile()
    return nc


def make_in_maps(x, gamma, mem_kv, w_qkv, w_out):
    """Host-side sharding: 8 in_maps, core = batch*4 + head_pair."""
    x = np.asarray(x, np.float32)
    gamma = np.asarray(gamma, np.float32)
    mem_kv = np.asarray(mem_kv, np.float32)
    w_qkv = np.asarray(w_qkv, np.float32)
    w_out = np.asarray(w_out, np.float32)
    b, c, h, w = x.shape
    npos = h * w
    g1 = gamma.reshape(c) + 1.0
    weff = w_qkv * g1[None, :]

    in_maps = []
    for core in range(8):
        bi, g = core // 4, core % 4
        h0, h1 = 2 * g, 2 * g + 1
        cols = []
        for blk in (0, HEADS * DIM_HEAD, 2 * HEADS * DIM_HEAD):
            for hh in (h0, h1):
                cols.append(weff[blk + DIM_HEAD * hh: blk + DIM_HEAD * (hh + 1)])
        wqkvT = np.zeros((c, 512), np.float32)
        wqkvT[:, 0:384] = np.concatenate(cols, 0).T
        wqkvT[0:128, 384] = 1.0  # ones column for the sumsq matmul
        npc = npos // 128
        kkinit = np.zeros((128, 128), np.float32)
        kkinit[0:64, 0:4] = mem_kv[0, h0].T
        kkinit[64:128, 0:4] = mem_kv[0, h1].T
        vvtinit = np.zeros((128, npc + 1, 130), np.float32)
        vvtinit[:, 0:npc, 64] = 1.0
        vvtinit[:, 0:npc, 129] = 1.0
        vvtinit[0:4, npc, 0:64] = mem_kv[1, h0]
        vvtinit[0:4, npc, 64] = 1.0
        vvtinit[0:4, npc, 65:129] = mem_kv[1, h1]
        vvtinit[0:4, npc, 129] = 1.0
        wout = np.zeros((64, 512), np.float32)
        wout[:, 0:256] = w_out[:, DIM_HEAD * h0: DIM_HEAD * (h0 + 1)].T
        wout[:, 256:512] = w_out[:, DIM_HEAD * h1: DIM_HEAD * (h1 + 1)].T
        in_maps.append({
            "x_sh": np.ascontiguousarray(x[bi].reshape(c, npos)),
            "wqkvT": round_f32r(wqkvT),
            "kkinit": round_f32r(kkinit),
            "vvtinit": round_f32r(vvtinit),
            "wout": round_f32r(wout),
        })
    return in_maps


_NC_CACHE = {}


def _get_nc(NPOS=4096):
    if NPOS not in _NC_CACHE:
        _NC_CACHE[NPOS] = build_nc(NPOS=NPOS)
    return _NC_CACHE[NPOS]


def run_on_cores(x, gamma, mem_kv, w_qkv, w_out, **kwargs):
    nc = _get_nc()
    in_maps = make_in_maps(x, gamma, mem_kv, w_qkv, w_out)
    res = bass_utils.run_bass_kernel_spmd(nc, in_maps,
                                          core_ids=list(range(8)), **kwargs)
    b, c, h, w = np.asarray(x).shape
    ys = np.stack([res.results[core]["y"] for core in range(8)])
    y = ys.reshape(2, 4, c, h * w).sum(1).reshape(b, c, h, w)
    return y.astype(np.float32), res


def kernel(x, gamma, mem_kv, w_qkv, w_out):
    y, _ = run_on_cores(x, gamma, mem_kv, w_qkv, w_out)
    return y
